# revision 24
# baseline (speedup 1.0000x reference)
"""BitLinear (ternary group-quantized linear) Trainium2 Bass kernel.

Computes: w_q = groupwise_ternary_quantize(weight, group=128 along in_features)
          out = x @ w_q.T + bias
for x (4, 2048, 4096) f32, weight (16384, 4096) f32, bias (16384,) f32.

Sharding (tensor-parallel): weight rows (out_features) and bias sharded
8 ways (2048 rows/core); x replicated; each core computes its
(8192, 2048) output slice; host concatenates.

Host staging (layout prep only — all math happens on device):
  - x is staged K-major as bf16: xt[p, b, kg, m'] = x[b*MB+m', kg*128+p],
    so the matmul's stationary tiles load with one contiguous 8-16KB run
    per partition and no on-chip cast/transpose work at all.
  - bias pre-broadcast to [128, N] (as in the original baseline).
  - weights staged as raw f32 rows; quantization runs on device.

Per-core kernel:
  Phase Q (groupwise ternary quant, f32 threshold math = reference-exact):
    per [128 rows x 512 k] chunk: one DVE windowed abs-reduce -> gsum;
    thr/nthr/nscale derived straight from gsum (no chaining); then
    qa=(w>thr)*scale and qb=(w<-thr)*(-scale) as two fused 2x-rate DVE
    tensor_scalars (bf16 out), combined qa+qb on DVE, PE-transposed
    (identity matmul) and evicted on ACT into an SBUF-resident K-major
    cache of 4 strips [128, 32, 512] bf16 (16 MB wq stays on-chip).
  Phase M: one composable_matmul_tile_kernel over all 4 strips:
    kxm = direct DMA of the pre-staged bf16 xt tiles (SP queue, with
    lookahead); kxn = the SBUF wq cache; fp32 psum; bias added during
    psum->sbuf eviction on DVE; f32 out stores on the ACT HWDGE queue.
"""

import os
from contextlib import ExitStack

import ml_dtypes
import numpy as np

import concourse.bass as bass
import concourse.mybir as mybir
import concourse.tile as tile
from concourse import bacc
from concourse.bass import ds, ts
from concourse.bass_utils import run_bass_kernel_spmd
from concourse.kernels.tile_matmul import (
    ShapeInfo,
    composable_matmul_tile_kernel,
)
from concourse.masks import make_identity

F32 = mybir.dt.float32
BF16 = mybir.dt.bfloat16
P = 128
N_CORES = 8
M_FULL = 8192          # 4*2048 tokens
K = 4096               # in_features
N_OUT_FULL = 16384     # out_features
N = N_OUT_FULL // N_CORES  # 2048 out rows per core
KG = K // P            # 32 contraction groups of 128 (also the quant groups)
MB = 256               # m batch (token block) size
N_STRIP = 512          # kxn cache strip width (= matmul N_TILE)
QK = 512               # k-chunk for the quant pipeline


def build_kernel(
    tc: tile.TileContext,
    ctx: ExitStack,
    m_tokens: int,
    k_tile: int = 4096,
    kxm_bufs: int = 3,
    kxm_ahead: int = 2,
    psum_n_bufs: int = 3,
    temps_n_bufs: int = 2,
    q_bufs: int = 3,
    wf_bufs: int = 0,              # 0 = q_bufs
    qk: int = QK,                  # quant chunk width along K
    qs_bufs: int = 4,
    tp_bufs: int = 2,
    prewarm: int = 3,
    m_split: tuple = (1, 3),
    q_drain: int = 0,              # fixed quant chunks drained per m-batch (0=uniform)
    x_dma_engine: str = "sync",
    w_dma_engine: str = "scalar",
    out_dma_engine: str = "gpsimd",
    comb_engine: str = "vector",   # qa+qb combine (DVE: no Pool hop in the chain)
    evict_engine: str = "scalar",  # wq psum -> cache evict (pe mode)
    wq_mode: str = "pe",           # "pe" | "xbar" (SP-queue DMA transpose)
    _skip_q: bool = False,
    _q_probe: str = "full",        # "full" | "nochain" | "dmaonly" (timing probes)
):
    nc = tc.nc
    QK = qk
    nb_m = m_tokens // MB
    n_strips = N // N_STRIP  # 4
    rts_per_strip = N_STRIP // P
    ksub = k_tile // P
    k_tiles = K // k_tile

    xt_ap = nc.dram_tensor(
        "xt", [P, nb_m, KG, MB], BF16, kind="ExternalInput"
    ).ap()
    w_ap = nc.dram_tensor("w", [N, K], F32, kind="ExternalInput").ap()
    biasb_ap = nc.dram_tensor("biasb", [P, N], BF16, kind="ExternalInput").ap()
    out_ap = nc.dram_tensor("out", [m_tokens, N], BF16, kind="ExternalOutput").ap()

    const = ctx.enter_context(tc.tile_pool(name="const", bufs=1))
    cache_pool = ctx.enter_context(tc.tile_pool(name="kxncache", bufs=1))
    dram = ctx.enter_context(tc.tile_pool(name="dram", bufs=1, space="DRAM"))

    # K-major quantized-weight cache, SBUF resident: strip s holds out-rows
    # [512*s, 512*(s+1)) for all k: [p = k % 128, gk = k // 128, row]
    cache_strips = [
        cache_pool.tile([P, KG, N_STRIP], BF16, tag=f"kxnc{s}", name=f"kxnc{s}")
        for s in range(n_strips)
    ]

    biasb_sb = const.tile([P, N], BF16, tag="biasb")
    nc.sync.dma_start(biasb_sb[:], biasb_ap)

    if wq_mode == "pe":
        ident = const.tile([P, P], BF16, tag="ident")
        make_identity(nc, ident[:])
        tp_pool = ctx.enter_context(
            tc.tile_pool(name="tpsum", bufs=tp_bufs, space="PSUM")
        )
    wq_tiles = [
        dram.tile([N_STRIP, K], BF16, tag=f"wqd{s}", name=f"wqd{s}")
        for s in range(n_strips)
    ] if wq_mode == "dram" else None

    # ---------------- Phase M kxm machinery (emitted first: prewarm) -------
    kxm_pool = ctx.enter_context(tc.tile_pool(name="kxm", bufs=kxm_bufs))
    kcache = {}
    x_dma = getattr(nc, x_dma_engine)

    def emit_kxm_load(b, kt):
        t = kxm_pool.tile([P, ksub, MB], BF16, tag="xkxm", name="xkxm")
        x_dma.dma_start(
            t[:], xt_ap[:, b, ds(kt * ksub, ksub), :]
        )
        kcache[(b, kt)] = t

    # Deferred quant chunks, drained a few per m-batch from inside the
    # matmul's kxm producer so their DVE/Pool/ACT/PE-transpose work
    # interleaves with the matmul's psum evictions in every engine queue
    # (bulk-emitting them instead parks the evictions behind 200+ us of
    # quant and starves the PE).
    quant_queue = []

    def kxm_producer(nc_, md):
        b, kt = md.m_batch_idx, md.k_tile_idx
        if (b, kt) not in kcache:
            emit_kxm_load(b, kt)
        # keep kxm_ahead future tiles in flight
        idx = b * k_tiles + kt
        for j in range(idx + 1, idx + 1 + kxm_ahead):
            nb, nkt = divmod(j, k_tiles)
            if nb < nb_m and (nb, nkt) not in kcache:
                emit_kxm_load(nb, nkt)
        if kt == 0 and quant_queue:
            if q_drain > 0:
                take = q_drain
            else:
                take = -(-len(quant_queue) // max(1, nb_m - 1 - b))
            for _ in range(min(take, len(quant_queue))):
                emit_q_chunk(*quant_queue.pop(0))
        return kcache.pop((b, kt))

    # ---------------- Phase Q: groupwise ternary quantization -------------
    q_pool = ctx.enter_context(tc.tile_pool(name="qp", bufs=q_bufs))
    qsmall = ctx.enter_context(tc.tile_pool(name="qsmall", bufs=qs_bufs))
    w_dma = getattr(nc, w_dma_engine)
    comb = getattr(nc, comb_engine)
    evict = getattr(nc, evict_engine)

    def emit_q_chunk(s, rt, h):
        """Quantize chunk (row-tile rt, k-chunk h) into cache strip s."""
        gq = QK // P
        col = (rt % rts_per_strip) * P
        if True:
            if True:
                wf = q_pool.tile([P, gq, P], F32, tag="wf", name="wf",
                                 bufs=wf_bufs or q_bufs)
                w_dma.dma_start(wf[:], w_ap[ds(rt * P, P), ds(h * QK, QK)])
                if _q_probe == "dmaonly":
                    return
                # Per-group |w| sums in ONE DVE windowed reduce (innermost
                # axis, abs applied on the fly).
                gsum = qsmall.tile([P, gq, 1], F32, tag="gsum", name="gsum")
                nc.vector.tensor_reduce(
                    gsum[:, :, 0], wf[:], mybir.AxisListType.X,
                    mybir.AluOpType.add, apply_absolute_value=True,
                )
                # thr = 0.5*max(gsum/128, eps); scale = max(gsum/128, eps);
                # nthr/nscale negated via min. All derived straight from
                # gsum so none chains on another.
                thr = qsmall.tile([P, gq, 1], F32, tag="thr", name="thr")
                nc.vector.tensor_scalar(
                    thr[:], gsum[:], 0.5 / P, 0.5e-8,
                    op0=mybir.AluOpType.mult, op1=mybir.AluOpType.max,
                )
                scale = qsmall.tile([P, gq, 1], F32, tag="scale", name="scale")
                nc.vector.tensor_scalar(
                    scale[:], gsum[:], 1.0 / P, 1e-8,
                    op0=mybir.AluOpType.mult, op1=mybir.AluOpType.max,
                )
                nthr = qsmall.tile([P, gq, 1], F32, tag="nthr", name="nthr")
                nc.vector.tensor_scalar(
                    nthr[:], gsum[:], -0.5 / P, -0.5e-8,
                    op0=mybir.AluOpType.mult, op1=mybir.AluOpType.min,
                )
                nscale = qsmall.tile([P, gq, 1], F32, tag="nscale", name="nscale")
                nc.vector.tensor_scalar(
                    nscale[:], gsum[:], -1.0 / P, -1e-8,
                    op0=mybir.AluOpType.mult, op1=mybir.AluOpType.min,
                )
                # qa = (w > thr)*scale, qb = (w < -thr)*(-scale): strict
                # compares match the reference exactly; fused 2-op
                # tensor_scalar per group with [P,1] vector scalars.
                qa = q_pool.tile([P, gq, P], BF16, tag="qa", name="qa", bufs=q_bufs + 1)
                qb = q_pool.tile([P, gq, P], BF16, tag="qb", name="qb", bufs=q_bufs + 1)
                for g in range(gq):
                    nc.vector.tensor_scalar(
                        qa[:, g, :], wf[:, g, :],
                        thr[:, g, :], scale[:, g, :],
                        op0=mybir.AluOpType.is_gt,
                        op1=mybir.AluOpType.mult,
                    )
                    nc.vector.tensor_scalar(
                        qb[:, g, :], wf[:, g, :],
                        nthr[:, g, :], nscale[:, g, :],
                        op0=mybir.AluOpType.is_lt,
                        op1=mybir.AluOpType.mult,
                    )
                wqb = q_pool.tile([P, gq, P], BF16, tag="wqb", name="wqb", bufs=q_bufs + 1)
                comb.tensor_tensor(wqb[:], qa[:], qb[:], op=mybir.AluOpType.add)
                if _q_probe == "nochain":
                    return
                if wq_mode == "dram":
                    # Stage the row-major wqb chunk to DRAM scratch (SWDGE);
                    # after a row-tile's last chunk, one XBAR DMA-transpose
                    # (SP queue) reads it back K-major into the cache strip.
                    # The cache columns then have 4 single-DMA writers and
                    # the PE queue stays pure matmuls.
                    nc.gpsimd.dma_start(
                        wq_tiles[s][ds(col, P), ds(h * QK, QK)], wqb[:]
                    )
                    if h == K // QK - 1:
                        src = wq_tiles[s][ds(col, P), :].rearrange(
                            "f (po pi) -> f po pi", pi=P
                        )
                        nc.sync.dma_start_transpose(
                            cache_strips[s][:, :, ds(col, P)], src
                        )
                    return
                cdst = cache_strips[s][:, ds(h * gq, gq), ds(col, P)]
                if wq_mode == "xbar":
                    # SBUF->SBUF XBAR DMA-transpose on the SP HWDGE queue
                    # (NOT the ACT queue -- that corrupts nondeterministically
                    # on HW). Keeps the PE queue pure matmuls so the quant
                    # never gates the matmul stream.
                    nc.sync.dma_start_transpose(cdst, wqb[:])
                else:
                    # Transpose each [128 rows, 128 k] chunk on the PE and
                    # evict straight into the K-major SBUF cache.
                    ptile = tp_pool.tile([P, gq * P], BF16, tag="wtp", name="wtp")
                    for g in range(gq):
                        nc.tensor.transpose(
                            ptile[:, ds(g * P, P)], wqb[:, g, :], ident[:]
                        )
                    csrc = ptile[:].rearrange("p (g m) -> p g m", g=gq)
                    if evict_engine == "scalar":
                        nc.scalar.activation(
                            cdst, csrc, mybir.ActivationFunctionType.Copy
                        )
                    else:
                        evict.tensor_copy(cdst, csrc)

    # ---------------- Phase M: the matmul ---------------------------------
    out_q = getattr(nc, out_dma_engine)

    def run_call(strip_base, strips_in_call):
        def kxn_producer(nc_, md):
            assert md.n_tile == N_STRIP and md.n_batch_idx == 0
            s = strip_base + md.n_tile_idx
            return cache_strips[s][:, ts(md.k_tile_idx, md.k_subtiles), :]

        def mxn_consumer(nc_, sbuf_tile, md):
            # sbuf_tile [128 (m%128), m_subtiles (mo), n_tile]
            n_off = strip_base * N_STRIP + md.n_tile_idx * md.n_tile
            dst = out_ap[ds(md.m_batch_idx * MB, MB), ds(n_off, md.n_tile)]
            out_q.dma_start(
                dst.rearrange("(mo p) n -> p mo n", p=P), sbuf_tile[:]
            )

        def bias_reducer(nc_, psum, sbuf, md):
            off = (
                strip_base * N_STRIP
                + md.n_tile_idx * N_STRIP
                + md.n_subtile_idx * md.n_subtile
            )
            nc_.vector.tensor_tensor(
                out=sbuf[:, 0, :],
                in0=psum,
                in1=biasb_sb[:, ds(off, md.n_subtile)],
                op=mybir.AluOpType.add,
            )

        composable_matmul_tile_kernel(
            tc=tc,
            kxm_shape=ShapeInfo(pdims=((P, KG),), fdims=(MB,) * nb_m),
            kxn_shape=ShapeInfo(pdims=((P, KG),), fdims=(strips_in_call * N_STRIP,)),
            output_type=BF16,
            kxm_producer=kxm_producer,
            kxn_producer=kxn_producer,
            mxn_consumer=mxn_consumer,
            mxn_subtile_reducer=bias_reducer,
            MATMUL_FREE_DIM=512,
            MAX_TILE_SIZE=512,
            MAX_K_TILE_SIZE=k_tile,
            cache_tiles=True,
            temps_n_bufs=temps_n_bufs,
            psum_n_bufs=psum_n_bufs,
        )

    # Emission schedule: x loads first (DMA overlaps the quant prologue);
    # quantize strip 0 only (the prologue), then matmul it over ALL tokens
    # while strips 1-3 quantize a few chunks per m-batch via the producer
    # hook; then matmul strips 1-3. x tiles stream twice (134 MB) on the
    # otherwise idle SP queue.
    assert len(m_split) == 2 and sum(m_split) == n_strips
    for j in range(min(prewarm * k_tiles, kxm_bufs - 1, nb_m * k_tiles)):
        b, kt = divmod(j, k_tiles)
        emit_kxm_load(b, kt)
    if _skip_q or _q_probe != "full":
        for s in range(n_strips):
            nc.any.memset(cache_strips[s][:], 0.0)
    if _skip_q:
        pass
    else:
        for s in range(m_split[0]):
            for rt in range(s * rts_per_strip, (s + 1) * rts_per_strip):
                for h in range(K // QK):
                    emit_q_chunk(s, rt, h)
        for s in range(m_split[0], n_strips):
            for rt in range(s * rts_per_strip, (s + 1) * rts_per_strip):
                for h in range(K // QK):
                    quant_queue.append((s, rt, h))
    run_call(0, m_split[0])
    while quant_queue:
        emit_q_chunk(*quant_queue.pop(0))
    for j in range(min(kxm_ahead + 1, kxm_bufs, nb_m * k_tiles)):
        b, kt = divmod(j, k_tiles)
        if (b, kt) not in kcache:
            emit_kxm_load(b, kt)
    run_call(m_split[0], m_split[1])


def build_program(m_tokens: int = M_FULL, **kw):
    nc = bacc.Bacc(
        "TRN2",
        target_bir_lowering=False,
        debug=False,
        enable_asserts=False,
        num_devices=N_CORES,
    )
    with tile.TileContext(nc) as tc, ExitStack() as ctx:
        build_kernel(tc, ctx, m_tokens, **kw)
    nc.compile()
    return nc


_program_cache = {}


def _get_program(m_tokens: int):
    if m_tokens not in _program_cache:
        _program_cache[m_tokens] = build_program(m_tokens)
    return _program_cache[m_tokens]


def make_in_maps(x: np.ndarray, weight: np.ndarray, bias: np.ndarray):
    """Shard the full inputs for the 8 cores: replicate x (staged K-major
    bf16), split w/bias rows."""
    m_tokens = x.shape[0] * x.shape[1]
    nb = m_tokens // MB
    xf = x.reshape(m_tokens, K)
    # xt[p, b, kg, m'] = x[b*MB+m', kg*128+p]
    xt = np.ascontiguousarray(
        xf.astype(ml_dtypes.bfloat16).reshape(nb, MB, KG, P).transpose(3, 0, 2, 1)
    )
    in_maps = []
    for c in range(N_CORES):
        wsh = np.ascontiguousarray(weight[c * N:(c + 1) * N])
        bsh = bias[c * N:(c + 1) * N]
        biasb = np.ascontiguousarray(
            np.broadcast_to(bsh[None, :], (P, N)).astype(ml_dtypes.bfloat16)
        )
        in_maps.append({"xt": xt, "w": wsh, "biasb": biasb})
    return in_maps


def kernel(x: np.ndarray, weight: np.ndarray, bias: np.ndarray):
    nc = _get_program(x.shape[0] * x.shape[1])
    in_maps = make_in_maps(x, weight, bias)
    res = run_bass_kernel_spmd(nc, in_maps, core_ids=list(range(N_CORES)))
    out = np.concatenate([res.results[c]["out"] for c in range(N_CORES)], axis=1)
    kernel.last_results = res
    return out.reshape(x.shape[0], x.shape[1], N_OUT_FULL).astype(np.float32)


def time_kernel(x: np.ndarray, weight: np.ndarray, bias: np.ndarray, iters: int = 9):
    """Time the on-device NEFF execution with device-resident inputs.

    Stages the inputs on the devices once, then measures the marginal
    per-execution cost: batches of different depth are queued and the slope
    between depths cancels the fixed per-dispatch-round overhead of the
    tunnelled PJRT path. Donated output buffers are created ON DEVICE
    (jitted zeros) so no host->device traffic pollutes the timing.
    Returns (slope_seconds, out_full ndarray).
    """
    import time

    import jax
    import jax.numpy as jnp
    from jax.experimental.shard_map import shard_map
    from jax.sharding import Mesh, NamedSharding, PartitionSpec

    from concourse import bass2jax
    from concourse.bass2jax import _bass_exec_p, install_neuronx_cc_hook

    install_neuronx_cc_hook()
    nc = _get_program(x.shape[0] * x.shape[1])
    in_maps = make_in_maps(x, weight, bias)

    partition_name = (
        nc.partition_id_tensor.name if nc.partition_id_tensor else None
    )
    in_names, out_names, out_avals = [], [], []
    for alloc in nc.m.functions[0].allocations:
        if not isinstance(alloc, mybir.MemoryLocationSet):
            continue
        name = alloc.memorylocations[0].name
        if alloc.kind == "ExternalInput":
            if name != partition_name:
                in_names.append(name)
        elif alloc.kind == "ExternalOutput":
            shape = tuple(alloc.tensor_shape)
            dtype = mybir.dt.np(alloc.dtype)
            out_avals.append(jax.core.ShapedArray(shape, dtype))
            out_names.append(name)
    n_params = len(in_names)
    n_outs = len(out_avals)
    all_in_names = list(in_names) + list(out_names)
    if partition_name is not None:
        all_in_names.append(partition_name)
    donate = tuple(range(n_params, n_params + n_outs))

    def _body(*args):
        operands = list(args)
        if partition_name is not None:
            operands.append(bass2jax.partition_id_tensor())
        outs = _bass_exec_p.bind(
            *operands,
            out_avals=tuple(out_avals),
            in_names=tuple(all_in_names),
            out_names=tuple(out_names),
            lowering_input_output_aliases=(),
            sim_require_finite=True,
            sim_require_nnan=True,
            nc=nc,
        )
        return tuple(outs)

    devices = jax.devices()[:N_CORES]
    mesh = Mesh(np.asarray(devices), ("core",))
    shard = NamedSharding(mesh, PartitionSpec("core"))
    in_specs = (PartitionSpec("core"),) * (n_params + n_outs)
    out_specs = (PartitionSpec("core"),) * n_outs

    concat_in = [
        jax.device_put(
            np.concatenate(
                [np.asarray(in_maps[c][nm]) for c in range(N_CORES)], axis=0
            ),
            shard,
        )
        for nm in in_names
    ]

    abstract_args = [
        jax.ShapeDtypeStruct(a.shape, a.dtype, sharding=a.sharding)
        for a in concat_in
    ] + [
        jax.ShapeDtypeStruct((N_CORES * av.shape[0],) + tuple(av.shape[1:]),
                             av.dtype, sharding=shard)
        for av in out_avals
    ]
    sharded = bass2jax.fast_dispatch_compile(
        lambda: jax.jit(
            shard_map(_body, mesh=mesh, in_specs=in_specs,
                      out_specs=out_specs, check_rep=False),
            donate_argnums=donate,
            keep_unused=True,
        ).lower(*abstract_args).compile()
    )

    gen_zeros = jax.jit(
        lambda: tuple(
            jnp.zeros((N_CORES * av.shape[0],) + tuple(av.shape[1:]), av.dtype)
            for av in out_avals
        ),
        out_shardings=(shard,) * n_outs,
    )

    # Warm up (NEFF load etc.) — also the correctness-bearing execution.
    jax.block_until_ready(concat_in)
    out_arrs = sharded(*concat_in, *gen_zeros())
    jax.block_until_ready(out_arrs)

    def run_batch(n):
        zsets = [gen_zeros() for _ in range(n)]
        for zs in zsets:
            jax.block_until_ready(zs)
        t0 = time.perf_counter()
        outs = [sharded(*concat_in, *zs) for zs in zsets]
        jax.block_until_ready(outs)
        dt = time.perf_counter() - t0
        del outs
        return dt

    lo = int(os.environ.get("BENCH_LO", "1"))
    hi = int(os.environ.get("BENCH_HI", "33"))
    t_lo, t_hi = [], []
    for _ in range(iters):
        t_lo.append(run_batch(lo))
        t_hi.append(run_batch(hi))
        print(f"  b{lo}: {t_lo[-1]*1e3:.2f} ms  b{hi}: {t_hi[-1]*1e3:.2f} ms")
    # Median-of-batches slope: robust to one-sided tunnel jitter in either
    # direction (a single glitched batch must not set the reported number).
    med = (float(np.median(t_hi)) - float(np.median(t_lo))) / (hi - lo)
    mn = (min(t_hi) - min(t_lo)) / (hi - lo)
    print(f"  slope med: {med*1e6:.1f} us  min: {mn*1e6:.1f} us")
    best = med

    i_out = out_names.index("out")
    out = np.asarray(out_arrs[i_out]).reshape(N_CORES, x.shape[0] * x.shape[1], N)
    out_full = np.concatenate([out[c] for c in range(N_CORES)], axis=1)
    return best, out_full.reshape(x.shape[0], x.shape[1], N_OUT_FULL)


# revision 25
# speedup vs baseline: 1.0154x; 1.0154x over previous
"""BitLinear (ternary group-quantized linear) Trainium2 Bass kernel.

Computes: w_q = groupwise_ternary_quantize(weight, group=128 along in_features)
          out = x @ w_q.T + bias
for x (4, 2048, 4096) f32, weight (16384, 4096) f32, bias (16384,) f32.

Sharding (tensor-parallel): weight rows (out_features) and bias sharded
8 ways (2048 rows/core); x replicated; each core computes its
(8192, 2048) output slice; host concatenates.

Host staging (layout prep only — all math happens on device):
  - x is staged K-major as bf16: xt[p, b, kg, m'] = x[b*MB+m', kg*128+p],
    so the matmul's stationary tiles load with one contiguous 8-16KB run
    per partition and no on-chip cast/transpose work at all.
  - bias pre-broadcast to [128, N] (as in the original baseline).
  - weights staged as raw f32 rows; quantization runs on device.

Per-core kernel:
  Phase Q (groupwise ternary quant, f32 threshold math = reference-exact):
    per [128 rows x 512 k] chunk: one DVE windowed abs-reduce -> gsum;
    thr/nthr/nscale derived straight from gsum (no chaining); then
    qa=(w>thr)*scale and qb=(w<-thr)*(-scale) as two fused 2x-rate DVE
    tensor_scalars (bf16 out), combined qa+qb on DVE, PE-transposed
    (identity matmul) and evicted on ACT into an SBUF-resident K-major
    cache of 4 strips [128, 32, 512] bf16 (16 MB wq stays on-chip).
  Phase M: one composable_matmul_tile_kernel over all 4 strips:
    kxm = direct DMA of the pre-staged bf16 xt tiles (SP queue, with
    lookahead); kxn = the SBUF wq cache; fp32 psum; bias added during
    psum->sbuf eviction on DVE (bf16 out tiles); bf16 out stores on the
    SWDGE queue (host casts the gathered output back to f32).
"""

import os
from contextlib import ExitStack

import ml_dtypes
import numpy as np

import concourse.bass as bass
import concourse.mybir as mybir
import concourse.tile as tile
from concourse import bacc
from concourse.bass import ds, ts
from concourse.bass_utils import run_bass_kernel_spmd
from concourse.kernels.tile_matmul import (
    ShapeInfo,
    composable_matmul_tile_kernel,
)
from concourse.masks import make_identity

F32 = mybir.dt.float32
BF16 = mybir.dt.bfloat16
P = 128
N_CORES = 8
M_FULL = 8192          # 4*2048 tokens
K = 4096               # in_features
N_OUT_FULL = 16384     # out_features
N = N_OUT_FULL // N_CORES  # 2048 out rows per core
KG = K // P            # 32 contraction groups of 128 (also the quant groups)
MB = 256               # m batch (token block) size
N_STRIP = 512          # kxn cache strip width (= matmul N_TILE)
QK = 512               # k-chunk for the quant pipeline


def build_kernel(
    tc: tile.TileContext,
    ctx: ExitStack,
    m_tokens: int,
    k_tile: int = 4096,
    kxm_bufs: int = 3,
    kxm_ahead: int = 2,
    psum_n_bufs: int = 3,
    temps_n_bufs: int = 2,
    q_bufs: int = 3,
    wf_bufs: int = 0,              # 0 = q_bufs
    qk: int = QK,                  # quant chunk width along K
    qs_bufs: int = 4,
    tp_bufs: int = 2,
    prewarm: int = 3,
    m_split: tuple = (1, 3),
    q_drain: int = 0,              # fixed quant chunks drained per m-batch (0=uniform)
    x_dma_engine: str = "sync",
    w_dma_engine: str = "scalar",
    out_dma_engine: str = "gpsimd",
    comb_engine: str = "vector",   # qa+qb combine (DVE: no Pool hop in the chain)
    evict_engine: str = "scalar",  # wq psum -> cache evict (pe mode)
    wq_mode: str = "pe",           # "pe" | "xbar" (SP-queue DMA transpose)
    _skip_q: bool = False,
    _q_probe: str = "full",        # "full" | "nochain" | "dmaonly" (timing probes)
):
    nc = tc.nc
    QK = qk
    nb_m = m_tokens // MB
    n_strips = N // N_STRIP  # 4
    rts_per_strip = N_STRIP // P
    ksub = k_tile // P
    k_tiles = K // k_tile

    xt_ap = nc.dram_tensor(
        "xt", [P, nb_m, KG, MB], BF16, kind="ExternalInput"
    ).ap()
    w_ap = nc.dram_tensor("w", [N, K], F32, kind="ExternalInput").ap()
    biasb_ap = nc.dram_tensor("biasb", [P, N], BF16, kind="ExternalInput").ap()
    out_ap = nc.dram_tensor("out", [m_tokens, N], BF16, kind="ExternalOutput").ap()

    const = ctx.enter_context(tc.tile_pool(name="const", bufs=1))
    cache_pool = ctx.enter_context(tc.tile_pool(name="kxncache", bufs=1))
    dram = ctx.enter_context(tc.tile_pool(name="dram", bufs=1, space="DRAM"))

    # K-major quantized-weight cache, SBUF resident: strip s holds out-rows
    # [512*s, 512*(s+1)) for all k: [p = k % 128, gk = k // 128, row]
    cache_strips = [
        cache_pool.tile([P, KG, N_STRIP], BF16, tag=f"kxnc{s}", name=f"kxnc{s}")
        for s in range(n_strips)
    ]

    biasb_sb = const.tile([P, N], BF16, tag="biasb")
    nc.sync.dma_start(biasb_sb[:], biasb_ap)

    if wq_mode == "pe":
        ident = const.tile([P, P], BF16, tag="ident")
        make_identity(nc, ident[:])
        tp_pool = ctx.enter_context(
            tc.tile_pool(name="tpsum", bufs=tp_bufs, space="PSUM")
        )
    wq_tiles = [
        dram.tile([N_STRIP, K], BF16, tag=f"wqd{s}", name=f"wqd{s}")
        for s in range(n_strips)
    ] if wq_mode == "dram" else None

    # ---------------- Phase M kxm machinery (emitted first: prewarm) -------
    kxm_pool = ctx.enter_context(tc.tile_pool(name="kxm", bufs=kxm_bufs))
    kcache = {}
    x_dma = getattr(nc, x_dma_engine)

    def emit_kxm_load(b, kt):
        t = kxm_pool.tile([P, ksub, MB], BF16, tag="xkxm", name="xkxm")
        x_dma.dma_start(
            t[:], xt_ap[:, b, ds(kt * ksub, ksub), :]
        )
        kcache[(b, kt)] = t

    # Deferred quant chunks, drained a few per m-batch from inside the
    # matmul's kxm producer so their DVE/Pool/ACT/PE-transpose work
    # interleaves with the matmul's psum evictions in every engine queue
    # (bulk-emitting them instead parks the evictions behind 200+ us of
    # quant and starves the PE).
    quant_queue = []

    def kxm_producer(nc_, md):
        b, kt = md.m_batch_idx, md.k_tile_idx
        if (b, kt) not in kcache:
            emit_kxm_load(b, kt)
        # keep kxm_ahead future tiles in flight
        idx = b * k_tiles + kt
        for j in range(idx + 1, idx + 1 + kxm_ahead):
            nb, nkt = divmod(j, k_tiles)
            if nb < nb_m and (nb, nkt) not in kcache:
                emit_kxm_load(nb, nkt)
        if kt == 0 and quant_queue:
            if q_drain > 0:
                take = q_drain
            else:
                take = -(-len(quant_queue) // max(1, nb_m - 1 - b))
            for _ in range(min(take, len(quant_queue))):
                emit_q_chunk(*quant_queue.pop(0))
        return kcache.pop((b, kt))

    # ---------------- Phase Q: groupwise ternary quantization -------------
    q_pool = ctx.enter_context(tc.tile_pool(name="qp", bufs=q_bufs))
    qsmall = ctx.enter_context(tc.tile_pool(name="qsmall", bufs=qs_bufs))
    w_dma = getattr(nc, w_dma_engine)
    comb = getattr(nc, comb_engine)
    evict = getattr(nc, evict_engine)

    def emit_q_chunk(s, rt, h):
        """Quantize chunk (row-tile rt, k-chunk h) into cache strip s."""
        gq = QK // P
        col = (rt % rts_per_strip) * P
        if True:
            if True:
                wf = q_pool.tile([P, gq, P], F32, tag="wf", name="wf",
                                 bufs=wf_bufs or q_bufs)
                w_dma.dma_start(wf[:], w_ap[ds(rt * P, P), ds(h * QK, QK)])
                if _q_probe == "dmaonly":
                    return
                # Per-group |w| sums in ONE DVE windowed reduce (innermost
                # axis, abs applied on the fly).
                gsum = qsmall.tile([P, gq, 1], F32, tag="gsum", name="gsum")
                nc.vector.tensor_reduce(
                    gsum[:, :, 0], wf[:], mybir.AxisListType.X,
                    mybir.AluOpType.add, apply_absolute_value=True,
                )
                # thr = 0.5*max(gsum/128, eps); scale = max(gsum/128, eps);
                # nthr/nscale negated via min. All derived straight from
                # gsum so none chains on another.
                thr = qsmall.tile([P, gq, 1], F32, tag="thr", name="thr")
                nc.vector.tensor_scalar(
                    thr[:], gsum[:], 0.5 / P, 0.5e-8,
                    op0=mybir.AluOpType.mult, op1=mybir.AluOpType.max,
                )
                scale = qsmall.tile([P, gq, 1], F32, tag="scale", name="scale")
                nc.vector.tensor_scalar(
                    scale[:], gsum[:], 1.0 / P, 1e-8,
                    op0=mybir.AluOpType.mult, op1=mybir.AluOpType.max,
                )
                nthr = qsmall.tile([P, gq, 1], F32, tag="nthr", name="nthr")
                nc.vector.tensor_scalar(
                    nthr[:], gsum[:], -0.5 / P, -0.5e-8,
                    op0=mybir.AluOpType.mult, op1=mybir.AluOpType.min,
                )
                nscale = qsmall.tile([P, gq, 1], F32, tag="nscale", name="nscale")
                nc.vector.tensor_scalar(
                    nscale[:], gsum[:], -1.0 / P, -1e-8,
                    op0=mybir.AluOpType.mult, op1=mybir.AluOpType.min,
                )
                # qa = (w > thr)*scale, qb = (w < -thr)*(-scale): strict
                # compares match the reference exactly; fused 2-op
                # tensor_scalar per group with [P,1] vector scalars.
                qa = q_pool.tile([P, gq, P], BF16, tag="qa", name="qa", bufs=q_bufs + 1)
                qb = q_pool.tile([P, gq, P], BF16, tag="qb", name="qb", bufs=q_bufs + 1)
                for g in range(gq):
                    nc.vector.tensor_scalar(
                        qa[:, g, :], wf[:, g, :],
                        thr[:, g, :], scale[:, g, :],
                        op0=mybir.AluOpType.is_gt,
                        op1=mybir.AluOpType.mult,
                    )
                    nc.vector.tensor_scalar(
                        qb[:, g, :], wf[:, g, :],
                        nthr[:, g, :], nscale[:, g, :],
                        op0=mybir.AluOpType.is_lt,
                        op1=mybir.AluOpType.mult,
                    )
                wqb = q_pool.tile([P, gq, P], BF16, tag="wqb", name="wqb", bufs=q_bufs + 1)
                comb.tensor_tensor(wqb[:], qa[:], qb[:], op=mybir.AluOpType.add)
                if _q_probe == "nochain":
                    return
                if wq_mode == "dram":
                    # Stage the row-major wqb chunk to DRAM scratch (SWDGE);
                    # after a row-tile's last chunk, one XBAR DMA-transpose
                    # (SP queue) reads it back K-major into the cache strip.
                    # The cache columns then have 4 single-DMA writers and
                    # the PE queue stays pure matmuls.
                    nc.gpsimd.dma_start(
                        wq_tiles[s][ds(col, P), ds(h * QK, QK)], wqb[:]
                    )
                    if h == K // QK - 1:
                        src = wq_tiles[s][ds(col, P), :].rearrange(
                            "f (po pi) -> f po pi", pi=P
                        )
                        nc.sync.dma_start_transpose(
                            cache_strips[s][:, :, ds(col, P)], src
                        )
                    return
                cdst = cache_strips[s][:, ds(h * gq, gq), ds(col, P)]
                if wq_mode == "xbar":
                    # SBUF->SBUF XBAR DMA-transpose on the SP HWDGE queue
                    # (NOT the ACT queue -- that corrupts nondeterministically
                    # on HW). Keeps the PE queue pure matmuls so the quant
                    # never gates the matmul stream.
                    nc.sync.dma_start_transpose(cdst, wqb[:])
                else:
                    # Transpose each [128 rows, 128 k] chunk on the PE and
                    # evict straight into the K-major SBUF cache.
                    ptile = tp_pool.tile([P, gq * P], BF16, tag="wtp", name="wtp")
                    for g in range(gq):
                        nc.tensor.transpose(
                            ptile[:, ds(g * P, P)], wqb[:, g, :], ident[:]
                        )
                    csrc = ptile[:].rearrange("p (g m) -> p g m", g=gq)
                    if evict_engine == "scalar":
                        nc.scalar.activation(
                            cdst, csrc, mybir.ActivationFunctionType.Copy
                        )
                    else:
                        evict.tensor_copy(cdst, csrc)

    # ---------------- Phase M: the matmul ---------------------------------
    out_q = getattr(nc, out_dma_engine)

    def run_call(strip_base, strips_in_call):
        def kxn_producer(nc_, md):
            assert md.n_tile == N_STRIP and md.n_batch_idx == 0
            s = strip_base + md.n_tile_idx
            return cache_strips[s][:, ts(md.k_tile_idx, md.k_subtiles), :]

        def mxn_consumer(nc_, sbuf_tile, md):
            # sbuf_tile [128 (m%128), m_subtiles (mo), n_tile]
            n_off = strip_base * N_STRIP + md.n_tile_idx * md.n_tile
            dst = out_ap[ds(md.m_batch_idx * MB, MB), ds(n_off, md.n_tile)]
            out_q.dma_start(
                dst.rearrange("(mo p) n -> p mo n", p=P), sbuf_tile[:]
            )

        def bias_reducer(nc_, psum, sbuf, md):
            off = (
                strip_base * N_STRIP
                + md.n_tile_idx * N_STRIP
                + md.n_subtile_idx * md.n_subtile
            )
            nc_.vector.tensor_tensor(
                out=sbuf[:, 0, :],
                in0=psum,
                in1=biasb_sb[:, ds(off, md.n_subtile)],
                op=mybir.AluOpType.add,
            )

        composable_matmul_tile_kernel(
            tc=tc,
            kxm_shape=ShapeInfo(pdims=((P, KG),), fdims=(MB,) * nb_m),
            kxn_shape=ShapeInfo(pdims=((P, KG),), fdims=(strips_in_call * N_STRIP,)),
            output_type=BF16,
            kxm_producer=kxm_producer,
            kxn_producer=kxn_producer,
            mxn_consumer=mxn_consumer,
            mxn_subtile_reducer=bias_reducer,
            MATMUL_FREE_DIM=512,
            MAX_TILE_SIZE=512,
            MAX_K_TILE_SIZE=k_tile,
            cache_tiles=True,
            temps_n_bufs=temps_n_bufs,
            psum_n_bufs=psum_n_bufs,
        )

    # Emission schedule: x loads first (DMA overlaps the quant prologue);
    # quantize strip 0 only (the prologue), then matmul it over ALL tokens
    # while strips 1-3 quantize a few chunks per m-batch via the producer
    # hook; then matmul strips 1-3. x tiles stream twice (134 MB) on the
    # otherwise idle SP queue.
    assert len(m_split) == 2 and sum(m_split) == n_strips
    for j in range(min(prewarm * k_tiles, kxm_bufs - 1, nb_m * k_tiles)):
        b, kt = divmod(j, k_tiles)
        emit_kxm_load(b, kt)
    if _skip_q or _q_probe != "full":
        for s in range(n_strips):
            nc.any.memset(cache_strips[s][:], 0.0)
    if _skip_q:
        pass
    else:
        for s in range(m_split[0]):
            for rt in range(s * rts_per_strip, (s + 1) * rts_per_strip):
                for h in range(K // QK):
                    emit_q_chunk(s, rt, h)
        for s in range(m_split[0], n_strips):
            for rt in range(s * rts_per_strip, (s + 1) * rts_per_strip):
                for h in range(K // QK):
                    quant_queue.append((s, rt, h))
    run_call(0, m_split[0])
    while quant_queue:
        emit_q_chunk(*quant_queue.pop(0))
    for j in range(min(kxm_ahead + 1, kxm_bufs, nb_m * k_tiles)):
        b, kt = divmod(j, k_tiles)
        if (b, kt) not in kcache:
            emit_kxm_load(b, kt)
    run_call(m_split[0], m_split[1])


def build_program(m_tokens: int = M_FULL, **kw):
    nc = bacc.Bacc(
        "TRN2",
        target_bir_lowering=False,
        debug=False,
        enable_asserts=False,
        num_devices=N_CORES,
    )
    with tile.TileContext(nc) as tc, ExitStack() as ctx:
        build_kernel(tc, ctx, m_tokens, **kw)
    nc.compile()
    return nc


_program_cache = {}


def _get_program(m_tokens: int):
    if m_tokens not in _program_cache:
        _program_cache[m_tokens] = build_program(m_tokens)
    return _program_cache[m_tokens]


def make_in_maps(x: np.ndarray, weight: np.ndarray, bias: np.ndarray):
    """Shard the full inputs for the 8 cores: replicate x (staged K-major
    bf16), split w/bias rows."""
    m_tokens = x.shape[0] * x.shape[1]
    nb = m_tokens // MB
    xf = x.reshape(m_tokens, K)
    # xt[p, b, kg, m'] = x[b*MB+m', kg*128+p]
    xt = np.ascontiguousarray(
        xf.astype(ml_dtypes.bfloat16).reshape(nb, MB, KG, P).transpose(3, 0, 2, 1)
    )
    in_maps = []
    for c in range(N_CORES):
        wsh = np.ascontiguousarray(weight[c * N:(c + 1) * N])
        bsh = bias[c * N:(c + 1) * N]
        biasb = np.ascontiguousarray(
            np.broadcast_to(bsh[None, :], (P, N)).astype(ml_dtypes.bfloat16)
        )
        in_maps.append({"xt": xt, "w": wsh, "biasb": biasb})
    return in_maps


def kernel(x: np.ndarray, weight: np.ndarray, bias: np.ndarray):
    nc = _get_program(x.shape[0] * x.shape[1])
    in_maps = make_in_maps(x, weight, bias)
    res = run_bass_kernel_spmd(nc, in_maps, core_ids=list(range(N_CORES)))
    out = np.concatenate([res.results[c]["out"] for c in range(N_CORES)], axis=1)
    kernel.last_results = res
    return out.reshape(x.shape[0], x.shape[1], N_OUT_FULL).astype(np.float32)


def time_kernel(x: np.ndarray, weight: np.ndarray, bias: np.ndarray, iters: int = 9):
    """Time the on-device NEFF execution with device-resident inputs.

    Stages the inputs on the devices once, then measures the marginal
    per-execution cost: batches of different depth are queued and the slope
    between depths cancels the fixed per-dispatch-round overhead of the
    tunnelled PJRT path. Donated output buffers are created ON DEVICE
    (jitted zeros) so no host->device traffic pollutes the timing.
    Returns (slope_seconds, out_full ndarray).
    """
    import time

    import jax
    import jax.numpy as jnp
    from jax.experimental.shard_map import shard_map
    from jax.sharding import Mesh, NamedSharding, PartitionSpec

    from concourse import bass2jax
    from concourse.bass2jax import _bass_exec_p, install_neuronx_cc_hook

    install_neuronx_cc_hook()
    nc = _get_program(x.shape[0] * x.shape[1])
    in_maps = make_in_maps(x, weight, bias)

    partition_name = (
        nc.partition_id_tensor.name if nc.partition_id_tensor else None
    )
    in_names, out_names, out_avals = [], [], []
    for alloc in nc.m.functions[0].allocations:
        if not isinstance(alloc, mybir.MemoryLocationSet):
            continue
        name = alloc.memorylocations[0].name
        if alloc.kind == "ExternalInput":
            if name != partition_name:
                in_names.append(name)
        elif alloc.kind == "ExternalOutput":
            shape = tuple(alloc.tensor_shape)
            dtype = mybir.dt.np(alloc.dtype)
            out_avals.append(jax.core.ShapedArray(shape, dtype))
            out_names.append(name)
    n_params = len(in_names)
    n_outs = len(out_avals)
    all_in_names = list(in_names) + list(out_names)
    if partition_name is not None:
        all_in_names.append(partition_name)
    donate = tuple(range(n_params, n_params + n_outs))

    def _body(*args):
        operands = list(args)
        if partition_name is not None:
            operands.append(bass2jax.partition_id_tensor())
        outs = _bass_exec_p.bind(
            *operands,
            out_avals=tuple(out_avals),
            in_names=tuple(all_in_names),
            out_names=tuple(out_names),
            lowering_input_output_aliases=(),
            sim_require_finite=True,
            sim_require_nnan=True,
            nc=nc,
        )
        return tuple(outs)

    devices = jax.devices()[:N_CORES]
    mesh = Mesh(np.asarray(devices), ("core",))
    shard = NamedSharding(mesh, PartitionSpec("core"))
    in_specs = (PartitionSpec("core"),) * (n_params + n_outs)
    out_specs = (PartitionSpec("core"),) * n_outs

    concat_in = [
        jax.device_put(
            np.concatenate(
                [np.asarray(in_maps[c][nm]) for c in range(N_CORES)], axis=0
            ),
            shard,
        )
        for nm in in_names
    ]

    abstract_args = [
        jax.ShapeDtypeStruct(a.shape, a.dtype, sharding=a.sharding)
        for a in concat_in
    ] + [
        jax.ShapeDtypeStruct((N_CORES * av.shape[0],) + tuple(av.shape[1:]),
                             av.dtype, sharding=shard)
        for av in out_avals
    ]
    sharded = bass2jax.fast_dispatch_compile(
        lambda: jax.jit(
            shard_map(_body, mesh=mesh, in_specs=in_specs,
                      out_specs=out_specs, check_rep=False),
            donate_argnums=donate,
            keep_unused=True,
        ).lower(*abstract_args).compile()
    )

    gen_zeros = jax.jit(
        lambda: tuple(
            jnp.zeros((N_CORES * av.shape[0],) + tuple(av.shape[1:]), av.dtype)
            for av in out_avals
        ),
        out_shardings=(shard,) * n_outs,
    )

    # Warm up (NEFF load etc.) — also the correctness-bearing execution.
    jax.block_until_ready(concat_in)
    out_arrs = sharded(*concat_in, *gen_zeros())
    jax.block_until_ready(out_arrs)

    def run_batch(n):
        zsets = [gen_zeros() for _ in range(n)]
        for zs in zsets:
            jax.block_until_ready(zs)
        t0 = time.perf_counter()
        outs = [sharded(*concat_in, *zs) for zs in zsets]
        jax.block_until_ready(outs)
        dt = time.perf_counter() - t0
        del outs
        return dt

    lo = int(os.environ.get("BENCH_LO", "1"))
    hi = int(os.environ.get("BENCH_HI", "33"))
    t_lo, t_hi = [], []
    for _ in range(iters):
        t_lo.append(run_batch(lo))
        t_hi.append(run_batch(hi))
        print(f"  b{lo}: {t_lo[-1]*1e3:.2f} ms  b{hi}: {t_hi[-1]*1e3:.2f} ms")
    # Median-of-batches slope: robust to one-sided tunnel jitter in either
    # direction (a single glitched batch must not set the reported number).
    med = (float(np.median(t_hi)) - float(np.median(t_lo))) / (hi - lo)
    mn = (min(t_hi) - min(t_lo)) / (hi - lo)
    print(f"  slope med: {med*1e6:.1f} us  min: {mn*1e6:.1f} us")
    best = med

    i_out = out_names.index("out")
    out = np.asarray(out_arrs[i_out]).reshape(N_CORES, x.shape[0] * x.shape[1], N)
    out_full = np.concatenate([out[c] for c in range(N_CORES)], axis=1)
    return best, out_full.reshape(x.shape[0], x.shape[1], N_OUT_FULL)
